# revision 12
# baseline (speedup 1.0000x reference)
"""CRF loss kernel for Trainium2 (8 NeuronCores, data-parallel over batch).

Algorithm (per core, 64 sequences):
  Denominator (log-partition): exp-space recurrences in bf16, split
  meet-in-the-middle so the serial chain is S/2 long instead of S:
    forward  s_w = (Wf^T s_{w-1}) (*) exp(xf_w),   w = 1..M
    backward g_k = (Wb^T g_{k-1}) (*) exp(xb_k),   k = 1..S-1-M
  States stack two 52-row blocks (seqs 0:32 at rows 0:52, seqs 32:64 at
  rows 64:116) so one 116-wide matmul + one DVE multiply advances all 64
  sequences one step. Per 52-block: rows 0:50 = tag state, rows 50/51 =
  forward r (pickup) / a (accumulate) for sequences ending at w <= M,
  with host-packed gates; the backward block uses row 50 as an exp(end)
  injection carrier alive for t >= L, so B_{L-1} = exp(end) appears
  exactly once. The 1/82 per-step rescale is folded into the emission
  tiles host-side; the forward r/a carry is scaled by 82 inside Wf to
  cancel it. After the loops, B_M = Wb^T g_final (one matmul) and den is
  either r+a (L <= M, scale 82^-L) or sum_j s_M[j] B_M[j] (L > M, scale
  82^-(L-1)); both raw values go to the host, which picks per sequence
  and applies ln + L*ln(82).
  Numerator: the tag-only parts (transitions/start/end path score) are
  host-side O(B*S) gathers. The scores-touching emit sum runs on
  device: GpSimd elementwise oh*scores products per 128-timestep chunk
  (off the critical PE/DVE engines), PE ones-matmuls reduce over t with
  PSUM accumulation across chunks, and DVE tensor_reduce folds the tag
  axis per sequence.
"""

import os
import numpy as np
import ml_dtypes

import concourse.bass as bass
import concourse.bacc as bacc
import concourse.mybir as mybir
from concourse import tile
from concourse.bass_utils import run_bass_kernel_spmd

B, S, T = 512, 1024, 50
NCORES = 8
BL = B // NCORES  # 64
HB = BL // 2      # 32
P1 = T + 2        # 52: tag state + 2 extra rows per block
PB = 116          # two blocks: rows 0:52 and 64:116
M = 512           # meet point: forward covers L <= M, backward L > M
NEG = np.float32(-1e30)
LN82 = float(np.log(np.float64(82.0)))
LNC = -LN82

WCH = 32                 # windows per ring chunk
NCF = 17                 # forward chunks (windows 0..512 used)
NCB = 16                 # backward chunks (init + 511 steps)
NWIN = (NCF + NCB) * WCH  # 1056
NCH = 8                  # numerator t-chunks (128 rows each)
CW = 2 * T               # combo cols: oh | scores
QW = 8 * T               # emit psum quarter width (8 seqs x 50 tags)

TRACE = os.environ.get("CRF_TRACE") == "1"

_cached = {}


def _build_nc():
    f32 = mybir.dt.float32
    bf16 = mybir.dt.bfloat16
    AF = mybir.ActivationFunctionType
    OP = mybir.AluOpType

    nc = bacc.Bacc(None, target_bir_lowering=False)

    # ---- DRAM I/O ----
    d_sct = nc.dram_tensor("sct", [PB, NWIN, HB], f32, kind="ExternalInput")
    d_combo = nc.dram_tensor("combo", [2, NCH, 128, HB, CW], bf16,
                             kind="ExternalInput")
    d_ewt = nc.dram_tensor("ewt", [PB, 2 * PB], bf16, kind="ExternalInput")
    d_onesl = nc.dram_tensor("onesl", [PB, 2], f32, kind="ExternalInput")
    d_onesra = nc.dram_tensor("onesra", [PB, 2], bf16, kind="ExternalInput")
    d_ones128 = nc.dram_tensor("ones128", [128, 1], bf16,
                               kind="ExternalInput")

    d_num = nc.dram_tensor("o_num", [1, BL], f32, kind="ExternalOutput")
    d_den = nc.dram_tensor("o_den", [4, HB], f32, kind="ExternalOutput")

    with tile.TileContext(nc) as tc:
        with (
            tc.tile_pool(name="const", bufs=1) as cpool,
            tc.tile_pool(name="ring", bufs=4) as ring,
            tc.tile_pool(name="state", bufs=3) as spool,
            tc.tile_pool(name="work", bufs=2) as wpool,
            tc.tile_pool(name="ps_state", bufs=2, space="PSUM") as ps_state,
            tc.tile_pool(name="ps_em", bufs=1, space="PSUM") as ps_em,
        ):
            # ---- small constants + first ring chunks (SP DMA queue) ----
            ewt = cpool.tile([PB, 2 * PB], bf16)
            nc.sync.dma_start(ewt[:], d_ewt[:])
            onesl = cpool.tile([PB, 2], f32)
            nc.sync.dma_start(onesl[:], d_onesl[:])
            onesra = cpool.tile([PB, 2], bf16)
            nc.sync.dma_start(onesra[:], d_onesra[:])
            ones128 = cpool.tile([128, 1], bf16)
            nc.sync.dma_start(ones128[:], d_ones128[:])

            expl = {}

            def ensure_chunk(st, m):
                c = m + (0 if st == 0 else NCF)
                if (st, m) in expl or m >= (NCF if st == 0 else NCB):
                    return
                tl = ring.tile([PB, WCH, HB], f32, tag=f"ring{st}")
                nc.sync.dma_start(tl[:], d_sct[:, c * WCH:(c + 1) * WCH, :])
                nc.scalar.activation(tl[:], tl[:], AF.Exp)
                expl[(st, m)] = tl

            for st in range(2):
                for m in range(3):
                    ensure_chunk(st, m)

            # combo tiles on the ACT DMA queue (parallel to SP's sct queue)
            combos = {}
            for h in range(2):
                for ch in range(NCH):
                    ct = cpool.tile([128, HB, CW], bf16, tag=f"combo{h}{ch}",
                                    name=f"combo{h}{ch}")
                    nc.gpsimd.dma_start(ct[:], d_combo[h, ch][:])
                    combos[(h, ch)] = ct

            # ---- init states from window 0 of each stream ----
            states = []
            for st in range(2):
                s0 = spool.tile([PB, HB], bf16, tag=f"state{st}",
                                name="state")
                nc.scalar.copy(s0[:], expl[(st, 0)][:, 0, :])
                states.append(s0)

            # ---- numerator emit work queue ----
            emit_all = cpool.tile([1, BL], f32)
            prods = {}
            em_ps = {}
            num_ops = []

            def mk_prod(h, ch, q):
                def run():
                    pr = wpool.tile([128, 8, T], bf16, tag="prod",
                                    name="prod", bufs=6)
                    ct = combos[(h, ch)]
                    bs = slice(q * 8, q * 8 + 8)
                    nc.gpsimd.tensor_mul(pr[:], ct[:, bs, 0:T],
                                         ct[:, bs, T:CW])
                    prods[(h, ch, q)] = pr
                return run

            def mk_mm(h, ch, q):
                def run():
                    if ch == 0:
                        em_ps[(h, q)] = ps_em.tile([1, 8, T], f32,
                                                   tag=f"em{q}",
                                                   name="emps", bufs=1)
                    pr = prods.pop((h, ch, q))
                    nc.tensor.matmul(
                        em_ps[(h, q)][:], ones128[:],
                        pr[:, :, :],
                        start=(ch == 0), stop=(ch == NCH - 1),
                        skip_group_check=True)
                return run

            def mk_red(h, q):
                def run():
                    off = h * HB + q * 8
                    nc.vector.tensor_reduce(
                        emit_all[0:1, off:off + 8],
                        em_ps[(h, q)][:],
                        mybir.AxisListType.X, OP.add)
                return run

            prodq = [(h, ch, q) for h in range(2) for ch in range(NCH)
                     for q in range(4)]
            for pi in range(4):
                num_ops.append(mk_prod(*prodq[pi]))
            for h in range(2):
                for ch in range(NCH):
                    for q in range(4):
                        gi = (h * NCH + ch) * 4 + q
                        if gi + 4 < len(prodq):
                            num_ops.append(mk_prod(*prodq[gi + 4]))
                        num_ops.append(mk_mm(h, ch, q))
                for q in range(4):
                    num_ops.append(mk_red(h, q))

            num_i = 0

            def pump_num(k):
                nonlocal num_i
                for _ in range(k):
                    if num_i >= len(num_ops):
                        return
                    num_ops[num_i]()
                    num_i += 1

            # ---- recurrences: fwd w = 1..M, bwd k = 1..S-1-M ----
            PUMP_START = 12
            NB = S - 1 - M  # 511 backward steps
            for w in range(1, M + 1):
                for st in range(2):
                    if st == 1 and w > NB:
                        continue
                    m = w // WCH
                    ensure_chunk(st, m)
                    ensure_chunk(st, m + 1)
                    ensure_chunk(st, m + 2)
                    ps = ps_state.tile([PB, HB], f32, tag=f"ps{st}",
                                       name="stateps", bufs=2)
                    nc.tensor.matmul(ps[:], ewt[:, st * PB:st * PB + PB],
                                     states[st][:], skip_group_check=True)
                    ns = spool.tile([PB, HB], bf16, tag=f"state{st}",
                                    name="state")
                    nc.vector.scalar_tensor_tensor(
                        ns[:], ps[:], 1.0, expl[(st, m)][:, w % WCH, :],
                        OP.mult, OP.mult,
                    )
                    states[st] = ns
                if w >= PUMP_START and w % 7 < 2:
                    pump_num(1)

            pump_num(len(num_ops))

            nc.sync.dma_start(d_num[:], emit_all[:])

            # ---- denominator meet: B_M = Wb^T g_final; meet + r/a ----
            bm_ps = ps_state.tile([PB, HB], f32, tag="ps1", name="stateps",
                                  bufs=2)
            nc.tensor.matmul(bm_ps[:], ewt[:, PB:2 * PB], states[1][:],
                             skip_group_check=True)
            prod = cpool.tile([PB, HB], f32)
            nc.vector.scalar_tensor_tensor(
                prod[:], bm_ps[:], 1.0, states[0][:], OP.mult, OP.mult)
            mt_ps = ps_state.tile([2, HB], f32, tag="ps0", name="meetps")
            nc.tensor.matmul(mt_ps[:], onesl[:], prod[:],
                             skip_group_check=True)
            mt_sb = cpool.tile([2, HB], f32)
            nc.scalar.copy(mt_sb[:], mt_ps[:])
            nc.sync.dma_start(d_den[0:2, :], mt_sb[:])
            ra_ps = ps_state.tile([2, HB], f32, tag="ps0", name="raps")
            nc.tensor.matmul(ra_ps[:], onesra[:], states[0][:],
                             skip_group_check=True)
            ra_sb = cpool.tile([2, HB], f32)
            nc.scalar.copy(ra_sb[:], ra_ps[:])
            nc.sync.dma_start(d_den[2:4, :], ra_sb[:])

    nc.compile()
    nc.finalize()
    return nc


def _host_inputs(token_scores, tags, token_mask, transitions,
                 start_transitions, end_transitions):
    ts = np.ascontiguousarray(token_scores, dtype=np.float32)
    tg = np.asarray(tags).astype(np.int64)
    mk = np.asarray(token_mask).astype(np.float32)
    tr = np.asarray(transitions, dtype=np.float32)
    st = np.asarray(start_transitions, dtype=np.float32)
    en = np.asarray(end_transitions, dtype=np.float32)
    L_all = mk.sum(1).astype(np.int64)  # [B]

    # ---- host-side numerator tag path: trans + start + end (O(B*S)) ----
    pair = tr[tg[:, :-1], tg[:, 1:]].astype(np.float64)
    trans_sum = (pair * mk[:, 1:].astype(np.float64)).sum(1)
    last_tags = tg[np.arange(B), L_all - 1]
    host_num = (trans_sum + st.astype(np.float64)[tg[:, 0]]
                + en.astype(np.float64)[last_tags])

    # ---- shared (replicated) constants ----
    ef = np.zeros((P1, P1), np.float64)
    ef[0:T, 0:T] = np.exp(tr.astype(np.float64))
    ef[0:T, T] = np.exp(en.astype(np.float64))
    ef[T + 1, T + 1] = 82.0
    ef[T, T + 1] = 82.0
    eb = np.zeros((P1, P1), np.float64)
    eb[0:T, 0:T] = np.exp(tr.astype(np.float64)).T
    eb[T, 0:T] = np.exp(en.astype(np.float64))
    eb[T, T] = 1.0

    ewt = np.zeros((PB, 2 * PB), np.float64)
    for st_i, e in ((0, ef), (1, eb)):
        ewt[0:P1, st_i * PB:st_i * PB + P1] = e
        ewt[64:64 + P1, st_i * PB + 64:st_i * PB + 64 + P1] = e
    ewt = ewt.astype(ml_dtypes.bfloat16)

    onesl = np.zeros((PB, 2), np.float32)
    onesl[0:T, 0] = 1.0
    onesl[64:64 + T, 1] = 1.0
    onesra = np.zeros((PB, 2), np.float32)
    onesra[T:T + 2, 0] = 1.0
    onesra[64 + T:64 + T + 2, 1] = 1.0
    onesra = onesra.astype(ml_dtypes.bfloat16)
    ones128 = np.ones((128, 1), np.float32).astype(ml_dtypes.bfloat16)

    in_maps = []
    for r in range(NCORES):
        sl = slice(r * BL, (r + 1) * BL)
        tsc, tgc, mkc = ts[sl], tg[sl], mk[sl]
        L = L_all[sl]

        # ---- window tiles: [PB, NWIN, HB] log-multipliers ----
        sct = np.full((PB, NWIN, HB), NEG, np.float32)
        for blk in range(2):
            r0 = 64 * blk
            bs = slice(blk * HB, (blk + 1) * HB)
            logi = tsc[bs].transpose(2, 1, 0)            # [T, S, HB]
            Lb = L[bs]
            live = (np.arange(S)[None, :] < Lb[:, None]).T  # [S, HB]
            sct[r0:r0 + T, 0:M + 1, :] = np.where(
                live[None, 0:M + 1, :], logi[:, 0:M + 1, :] + np.float32(LNC),
                NEG)
            sct[r0:r0 + T, 0, :] = logi[:, 0, :] + st[:, None]
            gl = np.minimum(Lb, M)                        # gate at w == L
            sct[r0 + T, gl, np.arange(HB)] = np.where(Lb <= M,
                                                      np.float32(LNC), NEG)
            sct[r0 + T + 1, 1:M + 1, :] = np.float32(LNC)
            # backward: window NCF*WCH + k holds x for t = S - k; k=0 init
            boff = NCF * WCH
            sct[r0 + T, boff, :] = 0.0                    # carrier init
            kk = np.arange(1, S - M)                      # 1..511
            tt = S - kk
            livb = (tt[None, :] < Lb[:, None]).T          # [NB, HB]
            sct[r0:r0 + T, boff + 1:boff + S - M, :] = np.where(
                livb[None, :, :],
                logi[:, tt, :] + np.float32(LNC), NEG)
            sct[r0 + T, boff + 1:boff + S - M, :] = np.where(
                (tt[None, :] >= Lb[:, None]).T, np.float32(0.0), NEG)

        # ---- emit combo: [2, NCH, 128, HB, CW] = [oh | scores] ----
        oh = np.zeros((S, BL, T), np.float32)
        sidx = np.arange(S)
        bidx = np.arange(BL)
        oh[sidx[:, None], bidx[None, :], tgc[:, :].T] = 1.0
        oh *= mkc.T[:, :, None]
        combo = np.zeros((2, NCH, 128, HB, CW), np.float32)
        for h in range(2):
            bs = slice(h * HB, (h + 1) * HB)
            for ch in range(NCH):
                tt2 = slice(128 * ch, 128 * (ch + 1))
                combo[h, ch, :, :, 0:T] = oh[tt2, bs, :]
                combo[h, ch, :, :, T:CW] = \
                    tsc[bs, tt2, :].transpose(1, 0, 2)
        combo = combo.astype(ml_dtypes.bfloat16)

        in_maps.append({
            "sct": sct,
            "combo": combo,
            "ewt": ewt,
            "onesl": onesl,
            "onesra": onesra,
            "ones128": ones128,
        })
    return in_maps, L_all, host_num


def kernel(token_scores, tags, token_mask, transitions,
           start_transitions, end_transitions):
    if "nc" not in _cached:
        _cached["nc"] = _build_nc()
    nc = _cached["nc"]

    in_maps, L_all, host_num = _host_inputs(
        token_scores, tags, token_mask, transitions,
        start_transitions, end_transitions)
    res = run_bass_kernel_spmd(nc, in_maps, list(range(NCORES)), trace=TRACE)
    if TRACE and res.exec_time_ns is not None:
        _cached["exec_time_ns"] = res.exec_time_ns
        print(f"HW exec time: {res.exec_time_ns} ns")

    _cached['res'] = res
    total = np.float64(0.0)
    for r in range(NCORES):
        out = res.results[r]
        emit = out["o_num"].reshape(BL).astype(np.float64)
        num = emit + host_num[r * BL:(r + 1) * BL]
        den = out["o_den"].reshape(4, HB).astype(np.float64)
        meet = np.concatenate([den[0], den[1]])
        ra = np.concatenate([den[2], den[3]])
        L = L_all[r * BL:(r + 1) * BL]
        Lf = L.astype(np.float64)
        with np.errstate(divide="ignore"):
            denom = np.where(L <= M,
                             np.log(ra) + Lf * LN82,
                             np.log(meet) + (Lf - 1.0) * LN82)
        ll = num - denom
        total += np.float64(ll.sum(dtype=np.float64))
    loss = -(total / B)
    return np.array(loss, dtype=np.float32)


# revision 13
# speedup vs baseline: 1.0830x; 1.0830x over previous
"""CRF loss kernel for Trainium2 (8 NeuronCores, data-parallel over batch).

Algorithm (per core, 64 sequences):
  Denominator (log-partition): exp-space recurrences in bf16, split
  meet-in-the-middle so the serial chain is S/2 long instead of S:
    forward  s_w = (Wf^T s_{w-1}) (*) exp(xf_w),   w = 1..M
    backward g_k = (Wb^T g_{k-1}) (*) exp(xb_k),   k = 1..S-1-M
  States stack two 52-row blocks (seqs 0:32 at rows 0:52, seqs 32:64 at
  rows 64:116) so one 116-wide matmul + one DVE multiply advances all 64
  sequences one step. Per 52-block: rows 0:50 = tag state, rows 50/51 =
  forward r (pickup) / a (accumulate) for sequences ending at w <= M,
  with host-packed gates; the backward block uses row 50 as an exp(end)
  injection carrier alive for t >= L, so B_{L-1} = exp(end) appears
  exactly once. The 1/82 per-step rescale is folded into the emission
  tiles host-side; the forward r/a carry is scaled by 82 inside Wf to
  cancel it. After the loops, B_M = Wb^T g_final (one matmul) and den is
  either r+a (L <= M, scale 82^-L) or sum_j s_M[j] B_M[j] (L > M, scale
  82^-(L-1)); both raw values go to the host, which picks per sequence
  and applies ln + L*ln(82).
  Numerator: the tag-only parts (transitions/start/end path score) are
  host-side O(B*S) gathers. The scores-touching emit sum runs on
  device: GpSimd elementwise oh*scores products per 128-timestep chunk
  (off the critical PE/DVE engines), PE ones-matmuls reduce over t with
  PSUM accumulation across chunks, and DVE tensor_reduce folds the tag
  axis per sequence.
"""

import os
import numpy as np
import ml_dtypes

import concourse.bass as bass
import concourse.bacc as bacc
import concourse.mybir as mybir
from concourse import tile
from concourse.bass_utils import run_bass_kernel_spmd

B, S, T = 512, 1024, 50
NCORES = 8
BL = B // NCORES  # 64
HB = BL // 2      # 32
P1 = T + 2        # 52: tag state + 2 extra rows per block
PB = 116          # two blocks: rows 0:52 and 64:116
M = 512           # meet point: forward covers L <= M, backward L > M
NEG = np.float32(-1e30)
LN82 = float(np.log(np.float64(82.0)))
LNC = -LN82

WCH = 32                 # windows per ring chunk
NCF = 17                 # forward chunks (windows 0..512 used)
NCB = 16                 # backward chunks (init + 511 steps)
NWIN = (NCF + NCB) * WCH  # 1056
NCH = 8                  # numerator t-chunks (128 rows each)
CW = 2 * T               # combo cols: oh | scores
QW = 8 * T               # emit psum quarter width (8 seqs x 50 tags)

TRACE = os.environ.get("CRF_TRACE") == "1"

_cached = {}


def _build_nc():
    f32 = mybir.dt.float32
    bf16 = mybir.dt.bfloat16
    AF = mybir.ActivationFunctionType
    OP = mybir.AluOpType

    nc = bacc.Bacc(None, target_bir_lowering=False)

    # ---- DRAM I/O ----
    d_sct = nc.dram_tensor("sct", [NCF + NCB, PB, WCH * HB], f32,
                           kind="ExternalInput")
    d_combo = nc.dram_tensor("combo", [2, NCH, 128, HB, CW], bf16,
                             kind="ExternalInput")
    d_ewt = nc.dram_tensor("ewt", [PB, 2 * PB], bf16, kind="ExternalInput")
    d_onesl = nc.dram_tensor("onesl", [PB, 2], f32, kind="ExternalInput")
    d_onesra = nc.dram_tensor("onesra", [PB, 2], bf16, kind="ExternalInput")
    d_ones128 = nc.dram_tensor("ones128", [128, 1], bf16,
                               kind="ExternalInput")

    d_num = nc.dram_tensor("o_num", [1, BL], f32, kind="ExternalOutput")
    d_den = nc.dram_tensor("o_den", [4, HB], f32, kind="ExternalOutput")

    with tile.TileContext(nc) as tc:
        with (
            tc.tile_pool(name="const", bufs=1) as cpool,
            tc.tile_pool(name="ring", bufs=4) as ring,
            tc.tile_pool(name="state", bufs=3) as spool,
            tc.tile_pool(name="work", bufs=2) as wpool,
            tc.tile_pool(name="ps_state", bufs=2, space="PSUM") as ps_state,
            tc.tile_pool(name="ps_em", bufs=1, space="PSUM") as ps_em,
        ):
            # ---- small constants + first ring chunks (SP DMA queue) ----
            ewt = cpool.tile([PB, 2 * PB], bf16)
            nc.sync.dma_start(ewt[:], d_ewt[:])
            onesl = cpool.tile([PB, 2], f32)
            nc.sync.dma_start(onesl[:], d_onesl[:])
            onesra = cpool.tile([PB, 2], bf16)
            nc.sync.dma_start(onesra[:], d_onesra[:])
            ones128 = cpool.tile([128, 1], bf16)
            nc.sync.dma_start(ones128[:], d_ones128[:])

            expl = {}

            def ensure_chunk(st, m):
                c = m + (0 if st == 0 else NCF)
                if (st, m) in expl or m >= (NCF if st == 0 else NCB):
                    return
                tl = ring.tile([PB, WCH, HB], f32, tag=f"ring{st}")
                nc.sync.dma_start(tl[:], d_sct[c][:])
                nc.scalar.activation(tl[:], tl[:], AF.Exp)
                expl[(st, m)] = tl

            for st in range(2):
                for m in range(3):
                    ensure_chunk(st, m)

            combos = {}

            def mk_cdma(h, ch):
                def run():
                    ct = cpool.tile([128, HB, CW], bf16, tag=f"combo{h}{ch}",
                                    name=f"combo{h}{ch}")
                    nc.gpsimd.dma_start(ct[:], d_combo[h, ch][:])
                    combos[(h, ch)] = ct
                return run

            # ---- init states from window 0 of each stream ----
            states = []
            for st in range(2):
                s0 = spool.tile([PB, HB], bf16, tag=f"state{st}",
                                name="state")
                nc.scalar.copy(s0[:], expl[(st, 0)][:, 0, :])
                states.append(s0)

            # ---- numerator emit work queue ----
            emit_all = cpool.tile([1, BL], f32)
            prods = {}
            em_ps = {}
            num_ops = []

            def mk_prod(h, ch, q):
                def run():
                    pr = wpool.tile([128, 8, T], bf16, tag="prod",
                                    name="prod", bufs=6)
                    ct = combos[(h, ch)]
                    bs = slice(q * 8, q * 8 + 8)
                    nc.gpsimd.tensor_mul(pr[:], ct[:, bs, 0:T],
                                         ct[:, bs, T:CW])
                    prods[(h, ch, q)] = pr
                return run

            def mk_mm(h, ch, q):
                def run():
                    if ch == 0:
                        em_ps[(h, q)] = ps_em.tile([1, 8, T], f32,
                                                   tag=f"em{q}",
                                                   name="emps", bufs=1)
                    pr = prods.pop((h, ch, q))
                    nc.tensor.matmul(
                        em_ps[(h, q)][:], ones128[:],
                        pr[:, :, :],
                        start=(ch == 0), stop=(ch == NCH - 1),
                        skip_group_check=True)
                return run

            def mk_red(h, q):
                def run():
                    off = h * HB + q * 8
                    nc.vector.tensor_reduce(
                        emit_all[0:1, off:off + 8],
                        em_ps[(h, q)][:],
                        mybir.AxisListType.X, OP.add)
                return run

            prodq = [(h, ch, q) for h in range(2) for ch in range(NCH)
                     for q in range(4)]
            groups = [(h, ch) for h in range(2) for ch in range(NCH)]
            num_ops.append(mk_cdma(*groups[0]))
            num_ops.append(mk_cdma(*groups[1]))
            for pi in range(4):
                num_ops.append(mk_prod(*prodq[pi]))
            for h in range(2):
                for ch in range(NCH):
                    g = h * NCH + ch
                    if g + 2 < len(groups):
                        num_ops.append(mk_cdma(*groups[g + 2]))
                    for q in range(4):
                        gi = g * 4 + q
                        if gi + 4 < len(prodq):
                            num_ops.append(mk_prod(*prodq[gi + 4]))
                        num_ops.append(mk_mm(h, ch, q))
                for q in range(4):
                    num_ops.append(mk_red(h, q))

            num_i = 0

            def pump_num(k):
                nonlocal num_i
                for _ in range(k):
                    if num_i >= len(num_ops):
                        return
                    num_ops[num_i]()
                    num_i += 1

            # ---- recurrences: fwd w = 1..M, bwd k = 1..S-1-M ----
            PUMP_START = 12
            NB = S - 1 - M  # 511 backward steps
            for w in range(1, M + 1):
                for st in range(2):
                    if st == 1 and w > NB:
                        continue
                    m = w // WCH
                    ensure_chunk(st, m)
                    ensure_chunk(st, m + 1)
                    ensure_chunk(st, m + 2)
                    ps = ps_state.tile([PB, HB], f32, tag=f"ps{st}",
                                       name="stateps", bufs=2)
                    nc.tensor.matmul(ps[:], ewt[:, st * PB:st * PB + PB],
                                     states[st][:], skip_group_check=True)
                    ns = spool.tile([PB, HB], bf16, tag=f"state{st}",
                                    name="state")
                    nc.vector.scalar_tensor_tensor(
                        ns[:], ps[:], 1.0, expl[(st, m)][:, w % WCH, :],
                        OP.mult, OP.mult,
                    )
                    states[st] = ns
                if w >= PUMP_START and w % 7 < 2:
                    pump_num(1)

            pump_num(len(num_ops))

            nc.sync.dma_start(d_num[:], emit_all[:])

            # ---- denominator meet: B_M = Wb^T g_final; meet + r/a ----
            bm_ps = ps_state.tile([PB, HB], f32, tag="ps1", name="stateps",
                                  bufs=2)
            nc.tensor.matmul(bm_ps[:], ewt[:, PB:2 * PB], states[1][:],
                             skip_group_check=True)
            prod = cpool.tile([PB, HB], f32)
            nc.vector.scalar_tensor_tensor(
                prod[:], bm_ps[:], 1.0, states[0][:], OP.mult, OP.mult)
            mt_ps = ps_state.tile([2, HB], f32, tag="ps0", name="meetps")
            nc.tensor.matmul(mt_ps[:], onesl[:], prod[:],
                             skip_group_check=True)
            mt_sb = cpool.tile([2, HB], f32)
            nc.scalar.copy(mt_sb[:], mt_ps[:])
            nc.sync.dma_start(d_den[0:2, :], mt_sb[:])
            ra_ps = ps_state.tile([2, HB], f32, tag="ps0", name="raps")
            nc.tensor.matmul(ra_ps[:], onesra[:], states[0][:],
                             skip_group_check=True)
            ra_sb = cpool.tile([2, HB], f32)
            nc.scalar.copy(ra_sb[:], ra_ps[:])
            nc.sync.dma_start(d_den[2:4, :], ra_sb[:])

    nc.compile()
    nc.finalize()
    return nc


def _host_inputs(token_scores, tags, token_mask, transitions,
                 start_transitions, end_transitions):
    ts = np.ascontiguousarray(token_scores, dtype=np.float32)
    tg = np.asarray(tags).astype(np.int64)
    mk = np.asarray(token_mask).astype(np.float32)
    tr = np.asarray(transitions, dtype=np.float32)
    st = np.asarray(start_transitions, dtype=np.float32)
    en = np.asarray(end_transitions, dtype=np.float32)
    L_all = mk.sum(1).astype(np.int64)  # [B]

    # ---- host-side numerator tag path: trans + start + end (O(B*S)) ----
    pair = tr[tg[:, :-1], tg[:, 1:]].astype(np.float64)
    trans_sum = (pair * mk[:, 1:].astype(np.float64)).sum(1)
    last_tags = tg[np.arange(B), L_all - 1]
    host_num = (trans_sum + st.astype(np.float64)[tg[:, 0]]
                + en.astype(np.float64)[last_tags])

    # ---- shared (replicated) constants ----
    ef = np.zeros((P1, P1), np.float64)
    ef[0:T, 0:T] = np.exp(tr.astype(np.float64))
    ef[0:T, T] = np.exp(en.astype(np.float64))
    ef[T + 1, T + 1] = 82.0
    ef[T, T + 1] = 82.0
    eb = np.zeros((P1, P1), np.float64)
    eb[0:T, 0:T] = np.exp(tr.astype(np.float64)).T
    eb[T, 0:T] = np.exp(en.astype(np.float64))
    eb[T, T] = 1.0

    ewt = np.zeros((PB, 2 * PB), np.float64)
    for st_i, e in ((0, ef), (1, eb)):
        ewt[0:P1, st_i * PB:st_i * PB + P1] = e
        ewt[64:64 + P1, st_i * PB + 64:st_i * PB + 64 + P1] = e
    ewt = ewt.astype(ml_dtypes.bfloat16)

    onesl = np.zeros((PB, 2), np.float32)
    onesl[0:T, 0] = 1.0
    onesl[64:64 + T, 1] = 1.0
    onesra = np.zeros((PB, 2), np.float32)
    onesra[T:T + 2, 0] = 1.0
    onesra[64 + T:64 + T + 2, 1] = 1.0
    onesra = onesra.astype(ml_dtypes.bfloat16)
    ones128 = np.ones((128, 1), np.float32).astype(ml_dtypes.bfloat16)

    in_maps = []
    for r in range(NCORES):
        sl = slice(r * BL, (r + 1) * BL)
        tsc, tgc, mkc = ts[sl], tg[sl], mk[sl]
        L = L_all[sl]

        # ---- window tiles: [PB, NWIN, HB] log-multipliers ----
        sct = np.full((PB, NWIN, HB), NEG, np.float32)
        for blk in range(2):
            r0 = 64 * blk
            bs = slice(blk * HB, (blk + 1) * HB)
            logi = tsc[bs].transpose(2, 1, 0)            # [T, S, HB]
            Lb = L[bs]
            live = (np.arange(S)[None, :] < Lb[:, None]).T  # [S, HB]
            sct[r0:r0 + T, 0:M + 1, :] = np.where(
                live[None, 0:M + 1, :], logi[:, 0:M + 1, :] + np.float32(LNC),
                NEG)
            sct[r0:r0 + T, 0, :] = logi[:, 0, :] + st[:, None]
            gl = np.minimum(Lb, M)                        # gate at w == L
            sct[r0 + T, gl, np.arange(HB)] = np.where(Lb <= M,
                                                      np.float32(LNC), NEG)
            sct[r0 + T + 1, 1:M + 1, :] = np.float32(LNC)
            # backward: window NCF*WCH + k holds x for t = S - k; k=0 init
            boff = NCF * WCH
            sct[r0 + T, boff, :] = 0.0                    # carrier init
            kk = np.arange(1, S - M)                      # 1..511
            tt = S - kk
            livb = (tt[None, :] < Lb[:, None]).T          # [NB, HB]
            sct[r0:r0 + T, boff + 1:boff + S - M, :] = np.where(
                livb[None, :, :],
                logi[:, tt, :] + np.float32(LNC), NEG)
            sct[r0 + T, boff + 1:boff + S - M, :] = np.where(
                (tt[None, :] >= Lb[:, None]).T, np.float32(0.0), NEG)

        # ---- emit combo: [2, NCH, 128, HB, CW] = [oh | scores] ----
        oh = np.zeros((S, BL, T), np.float32)
        sidx = np.arange(S)
        bidx = np.arange(BL)
        oh[sidx[:, None], bidx[None, :], tgc[:, :].T] = 1.0
        oh *= mkc.T[:, :, None]
        combo = np.zeros((2, NCH, 128, HB, CW), np.float32)
        for h in range(2):
            bs = slice(h * HB, (h + 1) * HB)
            for ch in range(NCH):
                tt2 = slice(128 * ch, 128 * (ch + 1))
                combo[h, ch, :, :, 0:T] = oh[tt2, bs, :]
                combo[h, ch, :, :, T:CW] = \
                    tsc[bs, tt2, :].transpose(1, 0, 2)
        combo = combo.astype(ml_dtypes.bfloat16)

        sctc = np.ascontiguousarray(
            sct.reshape(PB, NCF + NCB, WCH, HB).transpose(1, 0, 2, 3)
            .reshape(NCF + NCB, PB, WCH * HB))
        in_maps.append({
            "sct": sctc,
            "combo": combo,
            "ewt": ewt,
            "onesl": onesl,
            "onesra": onesra,
            "ones128": ones128,
        })
    return in_maps, L_all, host_num


def kernel(token_scores, tags, token_mask, transitions,
           start_transitions, end_transitions):
    if "nc" not in _cached:
        _cached["nc"] = _build_nc()
    nc = _cached["nc"]

    in_maps, L_all, host_num = _host_inputs(
        token_scores, tags, token_mask, transitions,
        start_transitions, end_transitions)
    res = run_bass_kernel_spmd(nc, in_maps, list(range(NCORES)), trace=TRACE)
    if TRACE and res.exec_time_ns is not None:
        _cached["exec_time_ns"] = res.exec_time_ns
        print(f"HW exec time: {res.exec_time_ns} ns")

    _cached['res'] = res
    total = np.float64(0.0)
    for r in range(NCORES):
        out = res.results[r]
        emit = out["o_num"].reshape(BL).astype(np.float64)
        num = emit + host_num[r * BL:(r + 1) * BL]
        den = out["o_den"].reshape(4, HB).astype(np.float64)
        meet = np.concatenate([den[0], den[1]])
        ra = np.concatenate([den[2], den[3]])
        L = L_all[r * BL:(r + 1) * BL]
        Lf = L.astype(np.float64)
        with np.errstate(divide="ignore"):
            denom = np.where(L <= M,
                             np.log(ra) + Lf * LN82,
                             np.log(meet) + (Lf - 1.0) * LN82)
        ll = num - denom
        total += np.float64(ll.sum(dtype=np.float64))
    loss = -(total / B)
    return np.array(loss, dtype=np.float32)
